# revision 16
# baseline (speedup 1.0000x reference)
"""Distributed causal-attention kernel for TRN2 (8 NeuronCores).

Module: qkv = x@w_attn+b; q,k l2-normalized per head; scaled (8.0) causal
softmax attention; out = (attn@v reassembled)@w_proj + b_proj.
Shapes: x [2,2048,1024], 16 heads x 64 dim.

Sharding: tensor-parallel over heads (2 heads/core) for qkv+attention;
row-parallel for the output projection with each core owning 256 rows of
EACH batch.  Two 8-core AllToAlls (one per batch) redistribute the
per-head outputs: A2A(b0) fires when batch-0 attention is done and
overlaps batch-1 attention; A2A(b1) overlaps the batch-0 projection.

Key device-side choices (v4 — phase C is ACT-exp-bound, so engine queues
are laid out so batch-0's exps start as early as possible):
 - qkv lands in [seq, cols] layout; q,k are normalized with free-axis
   norms then transposed to [hd, seq] by XBAR DMA-transpose (no PE, no
   PSUM, no copy-back)
 - B1's PSUM evacuations for the batch-1 half run on GpSimd so the ACT
   and DVE queues reach batch-0 attention work without waiting for the
   batch-1 half of B1; rnorm for batch 1 and B2(b1) are emitted after
   C(b0) for the same reason
 - scores are computed transposed [k, q] with the two heads' matmuls
   adjacent: head0 lives in SBUF partitions 0-63, head1 in 64-127, so
   auto tile_positions (0,0)/(64,0) run both K=64 matmuls concurrently
   on PE row tiles
 - the exp'd tile is the AV matmul's MOVING operand: AV accumulates
   o^T[hd+1, q] over key tiles with the (v | ones) tile stationary.
   Row 64 of the accumulator is the softmax denominator.  scores/AVs
   are emitted in sub-batches (<=8 key tiles) to limit PE array mode
   switches between (64,128) scores and (128,128) AVs
 - softmax divide entirely off PE/ACT: GpSimd partition_broadcast fans
   the denominator row straight out of PSUM, DVE reciprocal ([64,512] —
   a [1,512] DVE recip runs on one lane and costs 3.4us) and one DVE
   multiply into the bf16 A2A staging tile
 - each PSUM accumulation group owns its bank (matmul start=True clears
   has_written for its whole 2KB zero region)
 - the A2A payload row is 64*shard + hd with heads packed in columns so
   rows stay 1KB and the receive side unpacks with plain strided DMAs
   into the K=128-packed projection layout; oc2 receive DMAs for batch b
   are emitted after batch b+1's staging DMAs so their wait on the
   collective can't stall the sync queue FIFO
 - the output projection packs source-core pairs into K=128: 4 matmuls
   per output tile; the batch-0 half runs under A2A(b1)
"""
import sys

if '/opt/trn_rl_repo' not in sys.path:
    sys.path.insert(0, '/opt/trn_rl_repo')

import numpy as np
import ml_dtypes

import concourse.bass as bass
import concourse.mybir as mybir
from concourse import bacc, tile
from concourse.bass import ts, ds
from concourse.bass_utils import run_bass_kernel_spmd

B, S, D, H = 2, 2048, 1024, 16
HD = D // H                 # 64
NCORES = 8
HPC = H // NCORES           # 2 heads per core
SEQT = 128
NT = (B * S) // SEQT        # 32 seq tiles (batch-major)
TPB = S // SEQT             # 16 tiles per batch
QSPAN = 512
NSPAN = S // QSPAN          # 4 q-spans per batch
ROWS = (B * S) // NCORES    # 512 output rows per core (256 per batch)
RPB = ROWS // B             # 256
KC = D // 128               # 8 contraction chunks
W3 = 3 * HPC * HD           # 384 qkv columns per core
BF = mybir.dt.bfloat16
F32 = mybir.dt.float32
HALF_LN8 = 1.0397207708399179  # 0.5*ln(8): folds the 8.0 score scale
AF = mybir.ActivationFunctionType
MUL = mybir.AluOpType.mult
SUB = 8                     # scores/AV sub-batch size (key tiles)


def build(dbg=False, with_bias=False):
    nc = bacc.Bacc("TRN2", target_bir_lowering=False, debug=False,
                   num_devices=NCORES)
    xt = nc.dram_tensor("xt", [D, B * S], BF, kind="ExternalInput")
    wq = nc.dram_tensor("wq", [D, W3], BF, kind="ExternalInput")
    ba = nc.dram_tensor("ba", [1, W3], BF, kind="ExternalInput")
    # proj weights pre-packed on host for K=128 contraction: partition
    # 64*t+r, free (h, p4, w) holds wp row 256*p4+128*t+64*h+r
    wp = nc.dram_tensor("wp", [128, HPC * (NCORES // 2) * D], BF,
                        kind="ExternalInput")
    bp = nc.dram_tensor("bp", [1, D], BF, kind="ExternalInput")
    # rows 0:256 = this core's batch-0 rows, 256:512 = batch-1 rows
    out = nc.dram_tensor("out", [ROWS, D], F32, kind="ExternalOutput")

    with tile.TileContext(nc) as tc:
        with tc.tile_pool(name="persist", bufs=1) as pp, \
             tc.tile_pool(name="dram", bufs=1, space="DRAM") as dram, \
             tc.tile_pool(name="work", bufs=4) as work, \
             tc.tile_pool(name="epool", bufs=20) as epool:

            # ---- persistent SBUF ----
            xt_sb = pp.tile([128, KC, B * S], BF, name="xt_sb")
            wq_sb = pp.tile([128, KC, W3], BF, name="wq_sb")
            wp_sb = [pp.tile([128, NCORES // 2, D], BF, name=f"wp_sb{h}")
                     for h in range(HPC)]
            ba_sb = pp.tile([1, W3], BF, name="ba_sb")
            bp_sb = pp.tile([1, D], BF, name="bp_sb")
            ones_sb = pp.tile([1, 128], BF, name="ones_sb")
            c_bias = pp.tile([128, 1], F32, name="c_bias")
            c_scale = pp.tile([128, 1], F32, name="c_scale")
            tri = pp.tile([128, 128], BF, name="tri")
            # q,k working copies (normalized in place) + batched norm stats
            qk_all = pp.tile([128, NT, 2 * HPC * HD], BF, name="qk_all")
            n2_all = pp.tile([128, NT, 2 * HPC], F32, name="n2_all")
            rn_all = pp.tile([128, NT, 2 * HPC], F32, name="rn_all")
            # qT/kT per batch: head0 rows 0:64, head1 rows 64:128
            QT = [pp.tile([128, S], BF, name=f"qt{b}") for b in range(B)]
            KT = [pp.tile([128, S], BF, name=f"kt{b}") for b in range(B)]
            # v in [seq, hd] layout, per head augmented with a ones column
            v_sb = pp.tile([128, NT, 2 * (HD + 1)], BF, name="v_sb")
            # o^T post-A2A, packed for K=128 proj: partitions 64*(p%2)+c,
            # pair block p//2; cols 0:256 batch0 rows, 256:512 batch1
            oc2 = [pp.tile([128, NCORES // 2, ROWS], BF, name=f"oc2{h}")
                   for h in range(HPC)]

            # per-batch A2A payload: shard p (dest core) = rows
            # [64p:64p+64) = hd channel; cols = (head, that core's 256
            # rows of this batch) so rows stay 1KB
            a2a_in = [dram.tile([64 * NCORES, HPC * RPB], BF,
                                name=f"a2a_in{b}") for b in range(B)]
            a2a_out = [dram.tile([64 * NCORES, HPC * RPB], BF,
                                 name=f"a2a_out{b}") for b in range(B)]

            # ---- constants.  First x chunk is small and first in the
            # queue so B1's first matmul starts ASAP ----
            nc.sync.dma_start(
                xt_sb[:, :, 0:256],
                xt[:, 0:256].rearrange("(kc p) w -> p kc w", p=128))
            nc.sync.dma_start(
                wq_sb[:], wq[:].rearrange("(kc p) w -> p kc w", p=128))
            nc.sync.dma_start(
                xt_sb[:, :, 256:1024],
                xt[:, 256:1024].rearrange("(kc p) w -> p kc w", p=128))
            for tq in range(1, 4):
                nc.sync.dma_start(
                    xt_sb[:, :, ds(1024 * tq, 1024)],
                    xt[:, ds(1024 * tq, 1024)].rearrange(
                        "(kc p) w -> p kc w", p=128))
            nc.sync.dma_start(ba_sb[:], ba[:])
            nc.sync.dma_start(bp_sb[:], bp[:])
            for h in range(HPC):
                nc.sync.dma_start(
                    wp_sb[h][:, :, :],
                    wp[:, ds(h * (NCORES // 2) * D, (NCORES // 2) * D)
                       ].rearrange("p (p4 w) -> p p4 w", p4=NCORES // 2))
            nc.gpsimd.memset(ones_sb[:], 1.0)
            nc.gpsimd.memset(c_bias[:], HALF_LN8)
            nc.gpsimd.memset(c_scale[:], -0.5)
            nc.gpsimd.memset(v_sb[:], 1.0)
            # tri[k, q] = 1 where q >= k (valid causal), else 0
            nc.gpsimd.memset(tri[:], 1.0)
            nc.gpsimd.affine_select(
                out=tri[:], in_=tri[:], compare_op=mybir.AluOpType.is_ge,
                fill=0.0, base=0, pattern=[[1, 128]], channel_multiplier=-1)

            # ---- phase B1 (per batch): qkv matmuls + norm stats.  The
            # batch-1 half is emitted AFTER C(b0) so every engine queue
            # reaches batch-0 attention work immediately; its PE matmuls
            # then overlap A2A(b0) and fill C(b0) stalls ----
            psB_holder = [None]

            def emit_b1(trange, qkv_pool, tag):
                for t in trange:
                    ps_full = qkv_pool.tile([128, QSPAN], F32, tag=tag,
                                            name=f"ps{t}")
                    ps = ps_full[:, 0:W3]
                    for kc in range(KC):
                        nc.tensor.matmul(ps, lhsT=xt_sb[:, kc, ts(t, 128)],
                                         rhs=wq_sb[:, kc, :],
                                         start=(kc == 0),
                                         stop=(not with_bias
                                               and kc == KC - 1))
                    if with_bias:
                        nc.tensor.matmul(ps, lhsT=ones_sb[:, 0:128],
                                         rhs=ba_sb[:], start=False,
                                         stop=True)
                    nc.scalar.copy(qk_all[:, t, :], ps_full[:, 0:256])
                    nc.vector.tensor_copy(
                        v_sb[:, t, :].rearrange(
                            "p (h e) -> p h e", e=HD + 1)[:, :, 0:HD],
                        ps_full[:, 256:384].rearrange("p (h e) -> p h e",
                                                      e=HD))
                    sq = work.tile([128, 2 * HPC * HD], BF, tag="sq",
                                   name=f"sq{t}")
                    nc.vector.tensor_mul(sq[:], qk_all[:, t, :],
                                         qk_all[:, t, :])
                    nc.vector.reduce_sum(
                        n2_all[:, t, :],
                        sq[:].rearrange("p (g e) -> p g e", e=HD),
                        axis=mybir.AxisListType.X)

            def emit_rnorm(lo, hi):
                nc.scalar.activation(
                    rn_all[:, lo:hi, :].rearrange("p a b -> p (a b)"),
                    n2_all[:, lo:hi, :].rearrange("p a b -> p (a b)"),
                    AF.Ln)
                nc.scalar.activation(
                    rn_all[:, lo:hi, :].rearrange("p a b -> p (a b)"),
                    rn_all[:, lo:hi, :].rearrange("p a b -> p (a b)"),
                    AF.Exp, scale=c_scale[:], bias=c_bias[:])

            ps_qkv_ctx = tc.tile_pool(name="ps_qkv", bufs=4, space="PSUM")
            ps_qkv = ps_qkv_ctx.__enter__()
            emit_b1(range(0, TPB), ps_qkv, "ps")
            emit_rnorm(0, TPB)
            ps_qkv_ctx.__exit__(None, None, None)
            psB_ctx = tc.tile_pool(name="psB", bufs=4, space="PSUM")
            psB = psB_ctx.__enter__()
            psC_ctx = tc.tile_pool(name="psC", bufs=4, space="PSUM")
            psC = psC_ctx.__enter__()

            # ---- phase B2 (per batch): normalize + XBAR DMA transpose --
            def emit_b2(trange):
                for t in trange:
                    b_, tt = divmod(t, TPB)
                    nc.vector.tensor_tensor(
                        qk_all[:, t, :].rearrange("p (g e) -> p g e", e=HD),
                        qk_all[:, t, :].rearrange("p (g e) -> p g e", e=HD),
                        rn_all[:, t, :, None].broadcast_to([128, 4, HD]),
                        op=MUL)
                    for src0, dst in ((0, QT[b_]), (128, KT[b_])):
                        nc.sync.dma_start_transpose(
                            dst[:, ts(tt, 128)],
                            qk_all[:, t, src0:src0 + 128])

            # a2a_in viewed [shard, hd, (head, col)]
            a2a_v = [a2a_in[b][:].rearrange("(s c) w -> s c w", c=64)
                     for b in range(B)]
            deferred_oc2 = []

            # ---- phase C (per batch): attention; heads zipped so the
            # two K=64 score matmuls run on concurrent PE row tiles ----
            def emit_attn(b_):
                for j in range(NSPAN):
                    nk = 4 * j + 4
                    avT = [psC.tile([HD + 1, QSPAN], F32, tag="av",
                                    name=f"avT{h}_{b_}_{j}")
                           for h in range(HPC)]
                    es = {}
                    for i0 in range(0, nk, SUB):
                        chunk = range(i0, min(i0 + SUB, nk))
                        for i in chunk:
                            d = i - 4 * j
                            c0 = max(d, 0)
                            for h in range(HPC):
                                sps = psB.tile([128, QSPAN], F32, tag="s",
                                               name=f"s{b_}_{j}_{h}_{i}")
                                nc.tensor.matmul(
                                    sps[:, 128 * c0:],
                                    lhsT=KT[b_][64 * h:64 * h + 64,
                                                ts(i, 128)],
                                    rhs=QT[b_][64 * h:64 * h + 64,
                                               ds(j * QSPAN + 128 * c0,
                                                  QSPAN - 128 * c0)],
                                    start=True, stop=True)
                                e = epool.tile([128, QSPAN], BF, tag="e",
                                               name=f"e{b_}_{j}_{h}_{i}")
                                nc.scalar.activation(e[:, 128 * c0:],
                                                     sps[:, 128 * c0:],
                                                     AF.Exp)
                                if d >= 0:
                                    nc.vector.tensor_tensor(
                                        e[:, 128 * d:128 * (d + 1)],
                                        e[:, 128 * d:128 * (d + 1)], tri[:],
                                        op=MUL)
                                es[h, i] = (e, c0)
                        for i in chunk:
                            for h in range(HPC):
                                e, c0 = es[h, i]
                                nc.tensor.matmul(
                                    avT[h][:, 128 * c0:],
                                    lhsT=v_sb[:, b_ * TPB + i,
                                              (HD + 1) * h:(HD + 1) * (h + 1)],
                                    rhs=e[:, 128 * c0:],
                                    start=(i == 0), stop=(i == nk - 1))

                    # softmax divide off PE/ACT; oT [64 hd, 512 q] splits
                    # 256/256 into the two dest shards of this span.
                    # (gpsimd custom ops cannot read PSUM, so DVE pulls
                    # the denominator row out first)
                    for h in range(HPC):
                        dn = work.tile([1, QSPAN], F32, tag="dn",
                                       name=f"dn{b_}_{j}_{h}")
                        nc.vector.tensor_copy(dn[:], avT[h][HD:HD + 1, :])
                        rdb = work.tile([64, QSPAN], F32, tag="rdb",
                                        name=f"rdb{b_}_{j}_{h}")
                        nc.gpsimd.partition_broadcast(rdb[:], dn[:])
                        nc.vector.reciprocal(rdb[:], rdb[:])
                        ot4 = work.tile([64, QSPAN], BF, tag="ot", bufs=4,
                                        name=f"ots{h}_{b_}_{j}")
                        nc.vector.tensor_tensor(ot4[:], rdb[:],
                                                avT[h][0:HD, :], op=MUL)
                        for s2 in range(2):
                            nc.sync.dma_start(
                                a2a_v[b_][2 * j + s2, :, ds(RPB * h, RPB)],
                                ot4[:, ds(RPB * s2, RPB)])

            def emit_a2a(b_):
                nc.gpsimd.collective_compute(
                    "AllToAll", mybir.AluOpType.bypass,
                    replica_groups=[list(range(NCORES))],
                    ins=[a2a_in[b_][:].opt()], outs=[a2a_out[b_][:].opt()])
                src = a2a_out[b_][:].rearrange(
                    "(p4 t c) (hh w) -> t hh c p4 w", p4=NCORES // 2, t=2,
                    hh=HPC)
                for h in range(HPC):
                    for t2 in range(2):
                        deferred_oc2.append(
                            (oc2[h][64 * t2:64 * t2 + 64, :,
                                    ds(RPB * b_, RPB)], src[t2, h]))

            emit_b2(range(0, TPB))
            emit_attn(0)
            emit_a2a(0)

            # batch-1 qkv + norms + B2, deferred so C(b0)'s work owns
            # every engine queue until now.  Its PSUM cycles through the
            # scores pool's banks (idle between the two batches).
            emit_b1(range(TPB, NT), psB, "s")
            emit_rnorm(TPB, NT)
            emit_b2(range(TPB, NT))
            emit_attn(1)
            # receive-side DMAs for A2A(b0), emitted behind batch-1's
            # staging DMAs so their wait can't stall the sync queue FIFO
            for dma_args in deferred_oc2:
                nc.sync.dma_start(*dma_args)
            deferred_oc2 = []
            emit_a2a(1)
            for dma_args in deferred_oc2:
                nc.sync.dma_start(*dma_args)

            # ---- phase D: projection; batch-0 rows run under A2A(b1).
            # Source-core pairs are packed in partitions for K=128 ----
            for b_ in range(B):
                for rt in (2 * b_, 2 * b_ + 1):
                    for half in range(2):
                        yps = psB.tile([128, 512], F32, tag="s",
                                       name=f"y0_{rt}_{half}")
                        for p4 in range(NCORES // 2):
                            nc.tensor.matmul(
                                yps[:], lhsT=oc2[0][:, p4, ts(rt, 128)],
                                rhs=wp_sb[0][:, p4, ds(half * 512, 512)],
                                start=(p4 == 0),
                                stop=(not with_bias
                                      and p4 == NCORES // 2 - 1))
                        if with_bias:
                            nc.tensor.matmul(
                                yps[:], lhsT=ones_sb[:, 0:128],
                                rhs=bp_sb[:, ds(half * 512, 512)],
                                start=False, stop=True)
                        ysb = work.tile([128, 512], F32, tag="y", bufs=4,
                                        name=f"ysb{rt}_{half}")
                        nc.vector.tensor_copy(ysb[:], yps[:])
                        yps2 = psB.tile([128, 512], F32, tag="s",
                                        name=f"y1_{rt}_{half}")
                        for p4 in range(NCORES // 2):
                            nc.tensor.matmul(
                                yps2[:], lhsT=oc2[1][:, p4, ts(rt, 128)],
                                rhs=wp_sb[1][:, p4, ds(half * 512, 512)],
                                start=(p4 == 0),
                                stop=(p4 == NCORES // 2 - 1))
                        nc.vector.tensor_tensor(ysb[:], ysb[:], yps2[:],
                                                op=mybir.AluOpType.add)
                        nc.sync.dma_start(
                            out[ts(rt, 128), ds(half * 512, 512)], ysb[:])

            psC_ctx.__exit__(None, None, None)
            psB_ctx.__exit__(None, None, None)

    nc.compile()
    return nc


_NC = None


def _get_nc(with_bias=False):
    global _NC
    if _NC is None or _NC[1] != with_bias:
        _NC = (build(with_bias=with_bias), with_bias)
    return _NC[0]


def make_in_maps(x, w_attn, b_attn, w_proj, b_proj):
    bf = ml_dtypes.bfloat16
    xt = np.ascontiguousarray(x.reshape(B * S, D).T).astype(bf)
    # pack wp rows 256*p4+128*t+64*h+r -> [partition 64*t+r, (h, p4, w)]
    wp_ = np.ascontiguousarray(
        w_proj.reshape(NCORES // 2, 2, HPC, 64, D)
        .transpose(1, 3, 2, 0, 4)
        .reshape(128, HPC * (NCORES // 2) * D)).astype(bf)
    bp_ = b_proj.reshape(1, D).astype(bf)
    in_maps = []
    for c in range(NCORES):
        sl = slice(128 * c, 128 * c + 128)
        wq_ = np.ascontiguousarray(np.concatenate(
            [w_attn[:, sl], w_attn[:, 1024:2048][:, sl],
             w_attn[:, 2048:3072][:, sl]], axis=1)).astype(bf)
        ba_ = np.concatenate(
            [b_attn[sl], b_attn[1024:2048][sl],
             b_attn[2048:3072][sl]]).reshape(1, W3).astype(bf)
        in_maps.append({"xt": xt, "wq": wq_, "ba": ba_, "wp": wp_, "bp": bp_})
    return in_maps


def gather_out(results):
    out = np.empty((B, S, D), np.float32)
    for c in range(NCORES):
        r = results[c]["out"]
        out[0, RPB * c:RPB * (c + 1), :] = r[0:RPB]
        out[1, RPB * c:RPB * (c + 1), :] = r[RPB:ROWS]
    return out


def kernel(x, w_attn, b_attn, w_proj, b_proj):
    with_bias = bool(np.any(b_attn) or np.any(b_proj))
    nc = _get_nc(with_bias=with_bias)
    in_maps = make_in_maps(np.asarray(x, np.float32), np.asarray(w_attn, np.float32),
                           np.asarray(b_attn, np.float32),
                           np.asarray(w_proj, np.float32),
                           np.asarray(b_proj, np.float32))
    res = run_bass_kernel_spmd(nc, in_maps, core_ids=list(range(NCORES)))
    return gather_out(res.results)


# revision 20
# speedup vs baseline: 1.0952x; 1.0952x over previous
"""Distributed causal-attention kernel for TRN2 (8 NeuronCores).

Module: qkv = x@w_attn+b; q,k l2-normalized per head; scaled (8.0) causal
softmax attention; out = (attn@v reassembled)@w_proj + b_proj.
Shapes: x [2,2048,1024], 16 heads x 64 dim.

Sharding: tensor-parallel over heads (2 heads/core) for qkv+attention;
row-parallel for the output projection with each core owning 256 rows of
EACH batch.  Two 8-core AllToAlls (one per batch) redistribute the
per-head outputs: A2A(b0) fires when batch-0 attention is done and
overlaps batch-1 attention; A2A(b1) overlaps the batch-0 projection.

Key device-side choices (v4 — phase C is ACT-exp-bound, so engine queues
are laid out so batch-0's exps start as early as possible):
 - qkv lands in [seq, cols] layout; q,k are normalized with free-axis
   norms then transposed to [hd, seq] by XBAR DMA-transpose (no PE, no
   PSUM, no copy-back)
 - B1's PSUM evacuations for the batch-1 half run on GpSimd so the ACT
   and DVE queues reach batch-0 attention work without waiting for the
   batch-1 half of B1; rnorm for batch 1 and B2(b1) are emitted after
   C(b0) for the same reason
 - scores are computed transposed [k, q] with the two heads' matmuls
   adjacent: head0 lives in SBUF partitions 0-63, head1 in 64-127, so
   auto tile_positions (0,0)/(64,0) run both K=64 matmuls concurrently
   on PE row tiles
 - the exp'd tile is the AV matmul's MOVING operand: AV accumulates
   o^T[hd+1, q] over key tiles with the (v | ones) tile stationary.
   Row 64 of the accumulator is the softmax denominator.  scores/AVs
   are emitted in sub-batches (<=8 key tiles) to limit PE array mode
   switches between (64,128) scores and (128,128) AVs
 - softmax divide entirely off PE/ACT: GpSimd partition_broadcast fans
   the denominator row straight out of PSUM, DVE reciprocal ([64,512] —
   a [1,512] DVE recip runs on one lane and costs 3.4us) and one DVE
   multiply into the bf16 A2A staging tile
 - each PSUM accumulation group owns its bank (matmul start=True clears
   has_written for its whole 2KB zero region)
 - the A2A payload row is 64*shard + hd with heads packed in columns so
   rows stay 1KB and the receive side unpacks with plain strided DMAs
   into the K=128-packed projection layout; oc2 receive DMAs for batch b
   are emitted after batch b+1's staging DMAs so their wait on the
   collective can't stall the sync queue FIFO
 - the output projection packs source-core pairs into K=128: 4 matmuls
   per output tile; the batch-0 half runs under A2A(b1)
"""
import sys

if '/opt/trn_rl_repo' not in sys.path:
    sys.path.insert(0, '/opt/trn_rl_repo')

import numpy as np
import ml_dtypes

import concourse.bass as bass
import concourse.mybir as mybir
from concourse import bacc, tile
from concourse.bass import ts, ds
from concourse.bass_utils import run_bass_kernel_spmd
from concourse.masks import make_identity

B, S, D, H = 2, 2048, 1024, 16
HD = D // H                 # 64
NCORES = 8
HPC = H // NCORES           # 2 heads per core
SEQT = 128
NT = (B * S) // SEQT        # 32 seq tiles (batch-major)
TPB = S // SEQT             # 16 tiles per batch
QSPAN = 512
NSPAN = S // QSPAN          # 4 q-spans per batch
ROWS = (B * S) // NCORES    # 512 output rows per core (256 per batch)
RPB = ROWS // B             # 256
KC = D // 128               # 8 contraction chunks
W3 = 3 * HPC * HD           # 384 qkv columns per core
BF = mybir.dt.bfloat16
F32 = mybir.dt.float32
HALF_LN8 = 1.0397207708399179  # 0.5*ln(8): folds the 8.0 score scale
AF = mybir.ActivationFunctionType
MUL = mybir.AluOpType.mult
SUB = 8                     # scores/AV sub-batch size (key tiles)


def build(dbg=False, with_bias=False):
    nc = bacc.Bacc("TRN2", target_bir_lowering=False, debug=False,
                   num_devices=NCORES)
    xt = nc.dram_tensor("xt", [D, B * S], BF, kind="ExternalInput")
    wq = nc.dram_tensor("wq", [D, W3], BF, kind="ExternalInput")
    ba = nc.dram_tensor("ba", [1, W3], BF, kind="ExternalInput")
    # proj weights pre-packed on host for K=128 contraction: partition
    # 64*t+r, free (h, p4, w) holds wp row 256*p4+128*t+64*h+r
    wp = nc.dram_tensor("wp", [128, HPC * (NCORES // 2) * D], BF,
                        kind="ExternalInput")
    bp = nc.dram_tensor("bp", [1, D], BF, kind="ExternalInput")
    # rows 0:256 = this core's batch-0 rows, 256:512 = batch-1 rows
    out = nc.dram_tensor("out", [ROWS, D], F32, kind="ExternalOutput")

    with tile.TileContext(nc) as tc:
        with tc.tile_pool(name="persist", bufs=1) as pp, \
             tc.tile_pool(name="dram", bufs=1, space="DRAM") as dram, \
             tc.tile_pool(name="work", bufs=4) as work, \
             tc.tile_pool(name="epool", bufs=20) as epool:

            # ---- persistent SBUF ----
            xt_sb = pp.tile([128, KC, B * S], BF, name="xt_sb")
            wq_sb = pp.tile([128, KC, W3], BF, name="wq_sb")
            wp_sb = [pp.tile([128, NCORES // 2, D], BF, name=f"wp_sb{h}")
                     for h in range(HPC)]
            ba_sb = pp.tile([1, W3], BF, name="ba_sb")
            bp_sb = pp.tile([1, D], BF, name="bp_sb")
            ones_sb = pp.tile([1, 128], BF, name="ones_sb")
            c_bias = pp.tile([128, 1], F32, name="c_bias")
            c_scale = pp.tile([128, 1], F32, name="c_scale")
            ident = pp.tile([128, 128], BF, name="ident")
            tri = pp.tile([128, 128], BF, name="tri")
            # q,k working copies (normalized in place) + batched norm stats
            qk_all = pp.tile([128, NT, 2 * HPC * HD], BF, name="qk_all")
            n2_all = pp.tile([128, NT, 2 * HPC], F32, name="n2_all")
            rn_all = pp.tile([128, NT, 2 * HPC], F32, name="rn_all")
            # qT/kT per batch: head0 rows 0:64, head1 rows 64:128
            QT = [pp.tile([128, S], BF, name=f"qt{b}") for b in range(B)]
            KT = [pp.tile([128, S], BF, name=f"kt{b}") for b in range(B)]
            # v in [seq, hd] layout, per head augmented with a ones column
            v_sb = pp.tile([128, NT, 2 * (HD + 1)], BF, name="v_sb")
            # o^T post-A2A, packed for K=128 proj: partitions 64*(p%2)+c,
            # pair block p//2; cols 0:256 batch0 rows, 256:512 batch1
            oc2 = [pp.tile([128, NCORES // 2, ROWS], BF, name=f"oc2{h}")
                   for h in range(HPC)]

            # per-batch A2A payload: shard p (dest core) = rows
            # [64p:64p+64) = hd channel; cols = (head, that core's 256
            # rows of this batch) so rows stay 1KB
            a2a_in = [dram.tile([64 * NCORES, HPC * RPB], BF,
                                name=f"a2a_in{b}") for b in range(B)]
            a2a_out = [dram.tile([64 * NCORES, HPC * RPB], BF,
                                 name=f"a2a_out{b}") for b in range(B)]

            # ---- constants.  First x chunk is small and first in the
            # queue so B1's first matmul starts ASAP ----
            nc.sync.dma_start(
                xt_sb[:, :, 0:256],
                xt[:, 0:256].rearrange("(kc p) w -> p kc w", p=128))
            nc.sync.dma_start(
                wq_sb[:], wq[:].rearrange("(kc p) w -> p kc w", p=128))
            nc.sync.dma_start(
                xt_sb[:, :, 256:1024],
                xt[:, 256:1024].rearrange("(kc p) w -> p kc w", p=128))
            for tq in range(1, 4):
                nc.sync.dma_start(
                    xt_sb[:, :, ds(1024 * tq, 1024)],
                    xt[:, ds(1024 * tq, 1024)].rearrange(
                        "(kc p) w -> p kc w", p=128))
            nc.sync.dma_start(ba_sb[:], ba[:])
            nc.sync.dma_start(bp_sb[:], bp[:])
            for h in range(HPC):
                nc.sync.dma_start(
                    wp_sb[h][:, :, :],
                    wp[:, ds(h * (NCORES // 2) * D, (NCORES // 2) * D)
                       ].rearrange("p (p4 w) -> p p4 w", p4=NCORES // 2))
            nc.gpsimd.memset(ones_sb[:], 1.0)
            nc.gpsimd.memset(c_bias[:], HALF_LN8)
            nc.gpsimd.memset(c_scale[:], -0.5)
            nc.gpsimd.memset(v_sb[:], 1.0)
            make_identity(nc, ident[:])
            # tri[k, q] = 1 where q >= k (valid causal), else 0
            nc.gpsimd.memset(tri[:], 1.0)
            nc.gpsimd.affine_select(
                out=tri[:], in_=tri[:], compare_op=mybir.AluOpType.is_ge,
                fill=0.0, base=0, pattern=[[1, 128]], channel_multiplier=-1)

            # ---- phase B1 (per batch): qkv matmuls + norm stats.  The
            # batch-1 half is emitted AFTER C(b0) so every engine queue
            # reaches batch-0 attention work immediately; its PE matmuls
            # then overlap A2A(b0) and fill C(b0) stalls ----
            psB_holder = [None]

            def emit_b1(trange, qkv_pool, tag):
                for t in trange:
                    ps_full = qkv_pool.tile([128, QSPAN], F32, tag=tag,
                                            name=f"ps{t}")
                    ps = ps_full[:, 0:W3]
                    for kc in range(KC):
                        nc.tensor.matmul(ps, lhsT=xt_sb[:, kc, ts(t, 128)],
                                         rhs=wq_sb[:, kc, :],
                                         start=(kc == 0),
                                         stop=(not with_bias
                                               and kc == KC - 1))
                    if with_bias:
                        nc.tensor.matmul(ps, lhsT=ones_sb[:, 0:128],
                                         rhs=ba_sb[:], start=False,
                                         stop=True)
                    nc.scalar.copy(qk_all[:, t, :], ps_full[:, 0:256])
                    nc.vector.tensor_copy(
                        v_sb[:, t, :].rearrange(
                            "p (h e) -> p h e", e=HD + 1)[:, :, 0:HD],
                        ps_full[:, 256:384].rearrange("p (h e) -> p h e",
                                                      e=HD))
                    sq = work.tile([128, 2 * HPC * HD], BF, tag="sq",
                                   name=f"sq{t}")
                    nc.vector.tensor_mul(sq[:], qk_all[:, t, :],
                                         qk_all[:, t, :])
                    nc.vector.reduce_sum(
                        n2_all[:, t, :],
                        sq[:].rearrange("p (g e) -> p g e", e=HD),
                        axis=mybir.AxisListType.X)

            def emit_rnorm(lo, hi):
                nc.scalar.activation(
                    rn_all[:, lo:hi, :].rearrange("p a b -> p (a b)"),
                    n2_all[:, lo:hi, :].rearrange("p a b -> p (a b)"),
                    AF.Ln)
                nc.scalar.activation(
                    rn_all[:, lo:hi, :].rearrange("p a b -> p (a b)"),
                    rn_all[:, lo:hi, :].rearrange("p a b -> p (a b)"),
                    AF.Exp, scale=c_scale[:], bias=c_bias[:])

            ps_qkv_ctx = tc.tile_pool(name="ps_qkv", bufs=4, space="PSUM")
            ps_qkv = ps_qkv_ctx.__enter__()
            emit_b1(range(0, TPB), ps_qkv, "ps")
            emit_rnorm(0, TPB)
            ps_qkv_ctx.__exit__(None, None, None)
            psB_ctx = tc.tile_pool(name="psB", bufs=4, space="PSUM")
            psB = psB_ctx.__enter__()
            psC_ctx = tc.tile_pool(name="psC", bufs=4, space="PSUM")
            psC = psC_ctx.__enter__()

            # ---- phase B2 (per batch): normalize + PE transpose.  The
            # transposes borrow the scores pool's banks (bitcast view) —
            # sync-queue DMA transposes proved toxic: a scheduler-hoisted
            # collective wait blocks the whole FIFO under launch skew ----
            def emit_b2(trange):
                for t in trange:
                    b_, tt = divmod(t, TPB)
                    nc.vector.tensor_tensor(
                        qk_all[:, t, :].rearrange("p (g e) -> p g e", e=HD),
                        qk_all[:, t, :].rearrange("p (g e) -> p g e", e=HD),
                        rn_all[:, t, :, None].broadcast_to([128, 4, HD]),
                        op=MUL)
                    for src0, dst in ((0, QT[b_]), (128, KT[b_])):
                        trp_t = psB.tile([128, QSPAN], F32, tag="s",
                                         name=f"tr{t}_{src0}")
                        trp = trp_t[:].bitcast(BF)[:, 0:128]
                        nc.tensor.transpose(
                            trp, qk_all[:, t, src0:src0 + 128], ident[:])
                        nc.vector.tensor_copy(dst[:, ts(tt, 128)], trp)

            # a2a_in viewed [shard, hd, (head, col)]
            a2a_v = [a2a_in[b][:].rearrange("(s c) w -> s c w", c=64)
                     for b in range(B)]
            deferred_oc2 = []

            # ---- phase C (per batch): attention; heads zipped so the
            # two K=64 score matmuls run on concurrent PE row tiles ----
            def emit_attn(b_):
                for j in range(NSPAN):
                    nk = 4 * j + 4
                    avT = [psC.tile([HD + 1, QSPAN], F32, tag="av",
                                    name=f"avT{h}_{b_}_{j}")
                           for h in range(HPC)]
                    es = {}
                    for i0 in range(0, nk, SUB):
                        chunk = range(i0, min(i0 + SUB, nk))
                        for i in chunk:
                            d = i - 4 * j
                            c0 = max(d, 0)
                            for h in range(HPC):
                                sps = psB.tile([128, QSPAN], F32, tag="s",
                                               name=f"s{b_}_{j}_{h}_{i}")
                                nc.tensor.matmul(
                                    sps[:, 128 * c0:],
                                    lhsT=KT[b_][64 * h:64 * h + 64,
                                                ts(i, 128)],
                                    rhs=QT[b_][64 * h:64 * h + 64,
                                               ds(j * QSPAN + 128 * c0,
                                                  QSPAN - 128 * c0)],
                                    start=True, stop=True)
                                e = epool.tile([128, QSPAN], BF, tag="e",
                                               name=f"e{b_}_{j}_{h}_{i}")
                                nc.scalar.activation(e[:, 128 * c0:],
                                                     sps[:, 128 * c0:],
                                                     AF.Exp)
                                if d >= 0:
                                    nc.vector.tensor_tensor(
                                        e[:, 128 * d:128 * (d + 1)],
                                        e[:, 128 * d:128 * (d + 1)], tri[:],
                                        op=MUL)
                                es[h, i] = (e, c0)
                        for i in chunk:
                            for h in range(HPC):
                                e, c0 = es[h, i]
                                nc.tensor.matmul(
                                    avT[h][:, 128 * c0:],
                                    lhsT=v_sb[:, b_ * TPB + i,
                                              (HD + 1) * h:(HD + 1) * (h + 1)],
                                    rhs=e[:, 128 * c0:],
                                    start=(i == 0), stop=(i == nk - 1))

                    # softmax divide off PE/ACT; oT [64 hd, 512 q] splits
                    # 256/256 into the two dest shards of this span.
                    # (gpsimd custom ops cannot read PSUM, so DVE pulls
                    # the denominator row out first)
                    for h in range(HPC):
                        dn = work.tile([1, QSPAN], F32, tag="dn",
                                       name=f"dn{b_}_{j}_{h}")
                        nc.vector.tensor_copy(dn[:], avT[h][HD:HD + 1, :])
                        rdb = work.tile([64, QSPAN], F32, tag="rdb",
                                        name=f"rdb{b_}_{j}_{h}")
                        nc.gpsimd.partition_broadcast(rdb[:], dn[:])
                        nc.vector.reciprocal(rdb[:], rdb[:])
                        ot4 = work.tile([64, QSPAN], BF, tag="ot", bufs=4,
                                        name=f"ots{h}_{b_}_{j}")
                        nc.vector.tensor_tensor(ot4[:], rdb[:],
                                                avT[h][0:HD, :], op=MUL)
                        for s2 in range(2):
                            nc.sync.dma_start(
                                a2a_v[b_][2 * j + s2, :, ds(RPB * h, RPB)],
                                ot4[:, ds(RPB * s2, RPB)])

            def emit_a2a(b_):
                nc.gpsimd.collective_compute(
                    "AllToAll", mybir.AluOpType.bypass,
                    replica_groups=[list(range(NCORES))],
                    ins=[a2a_in[b_][:].opt()], outs=[a2a_out[b_][:].opt()])
                src = a2a_out[b_][:].rearrange(
                    "(p4 t c) (hh w) -> t hh c p4 w", p4=NCORES // 2, t=2,
                    hh=HPC)
                for h in range(HPC):
                    for t2 in range(2):
                        deferred_oc2.append(
                            (oc2[h][64 * t2:64 * t2 + 64, :,
                                    ds(RPB * b_, RPB)], src[t2, h]))

            emit_b2(range(0, TPB))
            emit_attn(0)
            emit_a2a(0)

            # batch-1 qkv + norms + B2, deferred so C(b0)'s work owns
            # every engine queue until now.  Its PSUM cycles through the
            # scores pool's banks (idle between the two batches).
            emit_b1(range(TPB, NT), psB, "s")
            emit_rnorm(TPB, NT)
            emit_b2(range(TPB, NT))
            emit_attn(1)
            # receive-side DMAs for A2A(b0), emitted behind batch-1's
            # staging DMAs so their wait can't stall the sync queue FIFO
            for dma_args in deferred_oc2:
                nc.sync.dma_start(*dma_args)
            deferred_oc2 = []
            emit_a2a(1)
            for dma_args in deferred_oc2:
                nc.sync.dma_start(*dma_args)

            # ---- phase D: projection; batch-0 rows run under A2A(b1).
            # Source-core pairs are packed in partitions for K=128 ----
            for b_ in range(B):
                for rt in (2 * b_, 2 * b_ + 1):
                    for half in range(2):
                        yps = psB.tile([128, 512], F32, tag="s",
                                       name=f"y0_{rt}_{half}")
                        for p4 in range(NCORES // 2):
                            nc.tensor.matmul(
                                yps[:], lhsT=oc2[0][:, p4, ts(rt, 128)],
                                rhs=wp_sb[0][:, p4, ds(half * 512, 512)],
                                start=(p4 == 0),
                                stop=(not with_bias
                                      and p4 == NCORES // 2 - 1))
                        if with_bias:
                            nc.tensor.matmul(
                                yps[:], lhsT=ones_sb[:, 0:128],
                                rhs=bp_sb[:, ds(half * 512, 512)],
                                start=False, stop=True)
                        ysb = work.tile([128, 512], F32, tag="y", bufs=4,
                                        name=f"ysb{rt}_{half}")
                        nc.vector.tensor_copy(ysb[:], yps[:])
                        yps2 = psB.tile([128, 512], F32, tag="s",
                                        name=f"y1_{rt}_{half}")
                        for p4 in range(NCORES // 2):
                            nc.tensor.matmul(
                                yps2[:], lhsT=oc2[1][:, p4, ts(rt, 128)],
                                rhs=wp_sb[1][:, p4, ds(half * 512, 512)],
                                start=(p4 == 0),
                                stop=(p4 == NCORES // 2 - 1))
                        nc.vector.tensor_tensor(ysb[:], ysb[:], yps2[:],
                                                op=mybir.AluOpType.add)
                        nc.sync.dma_start(
                            out[ts(rt, 128), ds(half * 512, 512)], ysb[:])

            psC_ctx.__exit__(None, None, None)
            psB_ctx.__exit__(None, None, None)

    nc.compile()
    return nc


_NC = None


def _get_nc(with_bias=False):
    global _NC
    if _NC is None or _NC[1] != with_bias:
        _NC = (build(with_bias=with_bias), with_bias)
    return _NC[0]


def make_in_maps(x, w_attn, b_attn, w_proj, b_proj):
    bf = ml_dtypes.bfloat16
    xt = np.ascontiguousarray(x.reshape(B * S, D).T).astype(bf)
    # pack wp rows 256*p4+128*t+64*h+r -> [partition 64*t+r, (h, p4, w)]
    wp_ = np.ascontiguousarray(
        w_proj.reshape(NCORES // 2, 2, HPC, 64, D)
        .transpose(1, 3, 2, 0, 4)
        .reshape(128, HPC * (NCORES // 2) * D)).astype(bf)
    bp_ = b_proj.reshape(1, D).astype(bf)
    in_maps = []
    for c in range(NCORES):
        sl = slice(128 * c, 128 * c + 128)
        wq_ = np.ascontiguousarray(np.concatenate(
            [w_attn[:, sl], w_attn[:, 1024:2048][:, sl],
             w_attn[:, 2048:3072][:, sl]], axis=1)).astype(bf)
        ba_ = np.concatenate(
            [b_attn[sl], b_attn[1024:2048][sl],
             b_attn[2048:3072][sl]]).reshape(1, W3).astype(bf)
        in_maps.append({"xt": xt, "wq": wq_, "ba": ba_, "wp": wp_, "bp": bp_})
    return in_maps


def gather_out(results):
    out = np.empty((B, S, D), np.float32)
    for c in range(NCORES):
        r = results[c]["out"]
        out[0, RPB * c:RPB * (c + 1), :] = r[0:RPB]
        out[1, RPB * c:RPB * (c + 1), :] = r[RPB:ROWS]
    return out


def kernel(x, w_attn, b_attn, w_proj, b_proj):
    with_bias = bool(np.any(b_attn) or np.any(b_proj))
    nc = _get_nc(with_bias=with_bias)
    in_maps = make_in_maps(np.asarray(x, np.float32), np.asarray(w_attn, np.float32),
                           np.asarray(b_attn, np.float32),
                           np.asarray(w_proj, np.float32),
                           np.asarray(b_proj, np.float32))
    res = run_bass_kernel_spmd(nc, in_maps, core_ids=list(range(NCORES)))
    return gather_out(res.results)
